# revision 6
# baseline (speedup 1.0000x reference)
"""Trainium2 Bass kernel for a Llama-style MoE layer (8 experts, top-2).

Strategy (8 NeuronCores, SPMD):
  - Expert-parallel: core e owns expert e's weights (w_gate/w_up/w_down[e]).
  - Host computes router logits once (tiny: [T,H]@[H,E]) purely to DECIDE
    dispatch; tokens routed to expert e are gathered, padded to capacity CP,
    and shipped transposed as xgt=[H,CP] to core e.
  - Device, per core, in one SPMD launch:
      * router for its 1/8 token shard: logits = x_shard @ router_w.T (fp32
        matmuls), top-2 via vector max/max_index, renormalized top-2 weights
        via sigmoid(l1-l2) — produces router_logits/top_i/top_w outputs.
      * expert SwiGLU MLP on its gathered tokens, entirely in
        [feature-partition, token-free] layout so no transposes are needed:
            gT[i,c] += wg[k,i].T @ xgt[k,c]   (accum over k: H-tiles)
            aT = silu(gT) * uT
            yT[h,c] += wd[k,h].T @ aT[k,c]    (accum over k: I-tiles)
        then yT scaled by the token's combine weight, DMA'd out.
  - Host scatters: out[t] = y[e1(t), pos1(t)] + y[e2(t), pos2(t)].

The MLP matmuls run as float32r (full fp32 storage; PE reduced-precision
single-pass mode, 1 cycle/row at N>=256 vs 4 for plain fp32).
"""

import sys

for _p in ("/opt/trn_rl_repo", "/root/.axon_site/_ro/trn_rl_repo"):
    if _p not in sys.path:
        sys.path.append(_p)

import numpy as np

from concourse import bacc, bass, mybir, tile
from concourse import bass_utils
from concourse.bass_utils import run_bass_kernel_spmd

# zero-egress container: don't ship NEFF/NTFF dirs to a bucket when tracing
bass_utils.upload_artifacts = lambda tmpdir: "local://" + tmpdir

B, S, H, I, E, K = 4, 2048, 2048, 5632, 8, 2
T = B * S
NCORES = 8
TPC = T // NCORES          # router tokens per core

F32 = mybir.dt.float32
F32R = mybir.dt.float32r
BF16 = mybir.dt.bfloat16
I32 = mybir.dt.int32
U32 = mybir.dt.uint32

MM_DT = F32R               # matmul compute dtype for the expert MLP
TRACE = False              # set by test harness to capture an NTFF profile
LAST_RESULT = None         # harness introspection: last BassKernelResults


def _chunks_for(cp: int) -> list[tuple[int, int]]:
    """Split capacity into free-dim chunks: full 512s plus a >=256 remainder."""
    assert cp % 128 == 0 and cp >= 256
    out = []
    off = 0
    while cp - off > 512:
        rem = cp - off
        if rem - 512 == 128:             # would leave a 128 tail: emit 384 now
            out.append((off, 384))
            off += 384
        else:
            out.append((off, 512))
            off += 512
    out.append((off, cp - off))
    assert sum(c for _, c in out) == cp
    assert all(c in (256, 384, 512) for _, c in out)
    return out


def round_capacity(n: int) -> int:
    cp = max(256, ((n + 127) // 128) * 128)
    if cp % 512 == 128:   # avoid a 128 tail chunk
        cp += 128
    return cp


def build_moe_program(cp, mm_dt=MM_DT, h=H, i_dim=I, tpc=TPC, compile=True):
    """One SPMD program: router shard + expert MLP at capacity cp."""
    chunks = _chunks_for(cp)
    kt_h = h // 128            # k-tiles over H
    mt_i = i_dim // 128        # m-tiles over I
    ht_h = h // 128            # h-tiles over H
    nc = bacc.Bacc(None, target_bir_lowering=False, debug=False)

    mdt = mm_dt                      # storage dtype for matmul operands
    xgt_d = nc.dram_tensor("xgt", [h, cp], mdt, kind="ExternalInput")
    wg_d = nc.dram_tensor("wg", [h, i_dim], mdt, kind="ExternalInput")
    wu_d = nc.dram_tensor("wu", [h, i_dim], mdt, kind="ExternalInput")
    wd_d = nc.dram_tensor("wd", [i_dim, h], mdt, kind="ExternalInput")
    sc_d = nc.dram_tensor("scale", [128, cp], F32, kind="ExternalInput")
    xrt_d = nc.dram_tensor("xrt", [h, tpc], F32, kind="ExternalInput")
    rwt_d = nc.dram_tensor("rwt", [h, E], F32, kind="ExternalInput")

    yt_d = nc.dram_tensor("yt", [h, cp], F32, kind="ExternalOutput")
    rlog_d = nc.dram_tensor("rlog", [tpc, E], F32, kind="ExternalOutput")
    rtopw_d = nc.dram_tensor("rtopw", [tpc, K], F32, kind="ExternalOutput")
    rtopi_d = nc.dram_tensor("rtopi", [tpc, K], I32, kind="ExternalOutput")

    with tile.TileContext(nc) as tc:
        # ---------------- router (tiny, fp32-exact) ----------------
        with (
            tc.tile_pool(name="rpool", bufs=2) as rpool,
            tc.tile_pool(name="rconst", bufs=1) as rconst,
            tc.tile_pool(name="rpsum", bufs=2, space="PSUM") as rpsum,
        ):
            zb = rconst.tile([128, 1], F32)
            nc.gpsimd.memset(zb[:], 0.0)
            rw_t = rconst.tile([128, kt_h, E], F32)
            nc.sync.dma_start(
                out=rw_t[:], in_=rwt_d.ap().rearrange("(k p) e -> p k e", p=128)
            )
            for mt in range(tpc // 128):
                xr_t = rpool.tile([128, kt_h, 128], F32, tag="xr")
                nc.sync.dma_start(
                    out=xr_t[:],
                    in_=xrt_d.ap()[:, mt * 128:(mt + 1) * 128].rearrange(
                        "(k p) t -> p k t", p=128
                    ),
                )
                ps = rpsum.tile([128, E], F32, tag="ps")
                for k in range(kt_h):
                    nc.tensor.matmul(
                        ps[:], xr_t[:, k, :], rw_t[:, k, :],
                        start=(k == 0), stop=(k == kt_h - 1),
                    )
                lg = rpool.tile([128, E], F32, tag="lg")
                nc.vector.tensor_copy(lg[:], ps[:])
                nc.sync.dma_start(out=rlog_d[mt * 128:(mt + 1) * 128, :], in_=lg[:])

                mx = rpool.tile([128, 8], F32, tag="mx")
                nc.vector.max(mx[:], lg[:])
                mi = rpool.tile([128, 8], U32, tag="mi")
                nc.vector.max_index(mi[:], mx[:], lg[:])

                d21 = rpool.tile([128, 1], F32, tag="d21")
                nc.vector.tensor_tensor(
                    d21[:], mx[:, 1:2], mx[:, 0:1], mybir.AluOpType.subtract
                )
                pw = rpool.tile([128, K], F32, tag="pw")
                # top_w = (sigma(l1-l2), sigma(l2-l1)) — exact softmax-renorm pair
                nc.scalar.activation(
                    pw[:, 0:1], d21[:], mybir.ActivationFunctionType.Sigmoid,
                    bias=zb[:], scale=-1.0,
                )
                nc.scalar.activation(
                    pw[:, 1:2], d21[:], mybir.ActivationFunctionType.Sigmoid,
                    bias=zb[:], scale=1.0,
                )
                nc.sync.dma_start(out=rtopw_d[mt * 128:(mt + 1) * 128, :], in_=pw[:])

                ti = rpool.tile([128, K], I32, tag="ti")
                nc.vector.tensor_copy(ti[:], mi[:, 0:K])
                nc.sync.dma_start(out=rtopi_d[mt * 128:(mt + 1) * 128, :], in_=ti[:])

        # ---------------- expert MLP ----------------
        with (
            tc.tile_pool(name="xgp", bufs=1) as xgp,
            tc.tile_pool(name="wgup", bufs=2) as wgup,
            tc.tile_pool(name="actp", bufs=1) as actp,
            tc.tile_pool(name="wdp", bufs=2) as wdp,
            tc.tile_pool(name="stage", bufs=2) as stage,
            tc.tile_pool(name="scp", bufs=1) as scp,
            tc.tile_pool(name="mpsum", bufs=2, space="PSUM") as mpsum,
        ):
            zb2 = scp.tile([128, 1], F32, tag="zb2")
            nc.gpsimd.memset(zb2[:], 0.0)
            for off, csz in chunks:
                xg = xgp.tile([128, kt_h, csz], mdt, tag="xg")
                nc.sync.dma_start(
                    out=xg[:],
                    in_=xgt_d.ap()[:, off:off + csz].rearrange(
                        "(k p) c -> p k c", p=128
                    ),
                )
                sct = scp.tile([128, csz], F32, tag="sc")
                nc.sync.dma_start(out=sct[:], in_=sc_d.ap()[:, off:off + csz])

                act = actp.tile([128, mt_i, csz], mdt, tag="act")
                for m in range(mt_i):
                    wgt = wgup.tile([128, kt_h, 128], mdt, tag="wg")
                    nc.sync.dma_start(
                        out=wgt[:],
                        in_=wg_d.ap()[:, m * 128:(m + 1) * 128].rearrange(
                            "(k p) m -> p k m", p=128
                        ),
                    )
                    wut = wgup.tile([128, kt_h, 128], mdt, tag="wu")
                    nc.sync.dma_start(
                        out=wut[:],
                        in_=wu_d.ap()[:, m * 128:(m + 1) * 128].rearrange(
                            "(k p) m -> p k m", p=128
                        ),
                    )
                    pg = mpsum.tile([128, csz], F32, tag="pg")
                    for k in range(kt_h):
                        nc.tensor.matmul(
                            pg[:], wgt[:, k, :], xg[:, k, :],
                            start=(k == 0), stop=(k == kt_h - 1),
                        )
                    pu = mpsum.tile([128, csz], F32, tag="pu")
                    for k in range(kt_h):
                        nc.tensor.matmul(
                            pu[:], wut[:, k, :], xg[:, k, :],
                            start=(k == 0), stop=(k == kt_h - 1),
                        )
                    st = stage.tile([128, csz], F32, tag="silu")
                    nc.scalar.activation(
                        st[:], pg[:], mybir.ActivationFunctionType.Silu, bias=zb2[:]
                    )
                    nc.vector.tensor_tensor(
                        act[:, m, :], st[:], pu[:], mybir.AluOpType.mult
                    )

                for hh in range(ht_h):
                    py = mpsum.tile([128, csz], F32, tag="py")
                    nhalf = 2 if mt_i % 2 == 0 else 1
                    for half in range(nhalf):
                        k0 = half * (mt_i // nhalf)
                        k1 = (half + 1) * (mt_i // nhalf) if half < nhalf - 1 else mt_i
                        wdt = wdp.tile([128, k1 - k0, 128], mdt, tag="wd")
                        nc.sync.dma_start(
                            out=wdt[:],
                            in_=wd_d.ap()[
                                k0 * 128:k1 * 128, hh * 128:(hh + 1) * 128
                            ].rearrange("(k p) m -> p k m", p=128),
                        )
                        for kk in range(k1 - k0):
                            k = k0 + kk
                            nc.tensor.matmul(
                                py[:], wdt[:, kk, :], act[:, k, :],
                                start=(k == 0), stop=(k == mt_i - 1),
                            )
                    yt_sb = stage.tile([128, csz], F32, tag="y")
                    nc.vector.tensor_tensor(
                        yt_sb[:], py[:], sct[:], mybir.AluOpType.mult
                    )
                    nc.sync.dma_start(
                        out=yt_d[hh * 128:(hh + 1) * 128, off:off + csz], in_=yt_sb[:]
                    )
    if compile:
        nc.compile()
    return nc


def _route_host(x: np.ndarray, router_w: np.ndarray):
    """Dispatch-only routing decisions (must order-match the fp32 reference)."""
    logits = x @ router_w.T                      # [T, E] fp32
    order = np.argsort(-logits, axis=1, kind="stable")
    top2 = order[:, :K].astype(np.int64)          # ties -> lower index, like jax
    l12 = np.take_along_axis(logits, top2, axis=1).astype(np.float64)
    d = l12[:, 0] - l12[:, 1]
    w1 = 1.0 / (1.0 + np.exp(-d))
    topw = np.stack([w1, 1.0 - w1], axis=1)       # renormalized top-2 weights
    return top2, topw


def kernel(hidden_states, router_w, w_gate, w_up, w_down):
    global LAST_RESULT
    x = np.ascontiguousarray(
        np.asarray(hidden_states, dtype=np.float32).reshape(T, H)
    )
    router_w = np.asarray(router_w, dtype=np.float32)
    w_gate = np.ascontiguousarray(np.asarray(w_gate, dtype=np.float32))
    w_up = np.ascontiguousarray(np.asarray(w_up, dtype=np.float32))
    w_down = np.ascontiguousarray(np.asarray(w_down, dtype=np.float32))

    top2, topw = _route_host(x, router_w)

    counts = np.bincount(top2.ravel(), minlength=E)
    cp = round_capacity(int(counts.max()))

    rows_per_e = []
    flat_idx = np.empty((T, K), dtype=np.int64)
    scales = np.zeros((E, cp), dtype=np.float32)
    for e in range(E):
        m0 = top2[:, 0] == e
        m1 = top2[:, 1] == e
        rows = np.nonzero(m0 | m1)[0]
        rows_per_e.append(rows)
        pos = np.arange(len(rows), dtype=np.int64)
        slot0 = m0[rows]                  # which slot routed each row here
        flat_idx[rows[slot0], 0] = e * cp + pos[slot0]
        flat_idx[rows[~slot0], 1] = e * cp + pos[~slot0]
        scales[e, :len(rows)] = np.where(
            slot0, topw[rows, 0], topw[rows, 1]
        ).astype(np.float32)

    rwt = np.ascontiguousarray(router_w.T)

    in_maps = []
    for e in range(NCORES):
        rows = rows_per_e[e]
        xgt = np.zeros((H, cp), dtype=np.float32)
        xgt[:, :len(rows)] = x[rows].T
        in_maps.append({
            "xgt": xgt,
            "wg": w_gate[e],
            "wu": w_up[e],
            "wd": w_down[e],
            "scale": np.ascontiguousarray(
                np.broadcast_to(scales[e], (128, cp))
            ),
            "xrt": np.ascontiguousarray(x[e * TPC:(e + 1) * TPC].T),
            "rwt": rwt,
        })

    nc = build_moe_program(cp)
    res = run_bass_kernel_spmd(nc, in_maps, list(range(NCORES)), trace=TRACE)
    LAST_RESULT = res
    results = res.results

    yflat = np.concatenate(
        [results[e]["yt"].T for e in range(E)], axis=0
    )  # [E*cp, H]
    out = yflat[flat_idx[:, 0]] + yflat[flat_idx[:, 1]]

    router_logits = np.concatenate(
        [results[c]["rlog"] for c in range(NCORES)], axis=0
    )
    top_i = np.concatenate(
        [results[c]["rtopi"] for c in range(NCORES)], axis=0
    ).astype(np.int32)
    top_w = np.concatenate(
        [results[c]["rtopw"] for c in range(NCORES)], axis=0
    )

    return (
        out.reshape(B, S, H),
        router_logits.reshape(B, S, E),
        top_i.reshape(B, S, K),
        top_w.reshape(B, S, K),
    )


# revision 8
# speedup vs baseline: 1.0274x; 1.0274x over previous
"""Trainium2 Bass kernel for a Llama-style MoE layer (8 experts, top-2).

Strategy (8 NeuronCores, SPMD):
  - Expert-parallel: core e owns expert e's weights (w_gate/w_up/w_down[e]).
  - Host computes router logits once (tiny: [T,H]@[H,E]) purely to DECIDE
    dispatch; tokens routed to expert e are gathered, padded to capacity CP,
    and shipped transposed as xgt=[H,CP] to core e.
  - Device, per core, in one SPMD launch:
      * router for its 1/8 token shard: logits = x_shard @ router_w.T (fp32
        matmuls), top-2 via vector max/max_index, renormalized top-2 weights
        via sigmoid(l1-l2) — produces router_logits/top_i/top_w outputs.
      * expert SwiGLU MLP on its gathered tokens, entirely in
        [feature-partition, token-free] layout so no transposes are needed:
            gT[i,c] += wg[k,i].T @ xgt[k,c]   (accum over k: H-tiles)
            aT = silu(gT) * uT
            yT[h,c] += wd[k,h].T @ aT[k,c]    (accum over k: I-tiles)
        then yT scaled by the token's combine weight, DMA'd out.
  - Host scatters: out[t] = y[e1(t), pos1(t)] + y[e2(t), pos2(t)].

The MLP matmuls run as float32r (full fp32 storage; PE reduced-precision
single-pass mode, 1 cycle/row at N>=256 vs 4 for plain fp32).
"""

import sys

for _p in ("/opt/trn_rl_repo", "/root/.axon_site/_ro/trn_rl_repo"):
    if _p not in sys.path:
        sys.path.append(_p)

import numpy as np
import ml_dtypes as _ml

from concourse import bacc, bass, mybir, tile
from concourse import bass_utils
from concourse.bass_utils import run_bass_kernel_spmd

# zero-egress container: don't ship NEFF/NTFF dirs to a bucket when tracing
bass_utils.upload_artifacts = lambda tmpdir: "local://" + tmpdir

B, S, H, I, E, K = 4, 2048, 2048, 5632, 8, 2
T = B * S
NCORES = 8
TPC = T // NCORES          # router tokens per core

F32 = mybir.dt.float32
F32R = mybir.dt.float32r
BF16 = mybir.dt.bfloat16
I32 = mybir.dt.int32
U32 = mybir.dt.uint32

MM_DT = F32R               # matmul compute dtype for the expert MLP
TRACE = False              # set by test harness to capture an NTFF profile
LAST_RESULT = None         # harness introspection: last BassKernelResults


def _chunks_for(cp: int) -> list[tuple[int, int]]:
    """Split capacity into free-dim chunks: full 512s plus a >=256 remainder."""
    assert cp % 128 == 0 and cp >= 256
    out = []
    off = 0
    while cp - off > 512:
        rem = cp - off
        if rem - 512 == 128:             # would leave a 128 tail: emit 384 now
            out.append((off, 384))
            off += 384
        else:
            out.append((off, 512))
            off += 512
    out.append((off, cp - off))
    assert sum(c for _, c in out) == cp
    assert all(c in (256, 384, 512) for _, c in out)
    return out


def round_capacity(n: int) -> int:
    cp = max(256, ((n + 127) // 128) * 128)
    if cp % 512 == 128:   # avoid a 128 tail chunk
        cp += 128
    return cp


def build_moe_program(cp, mm_dt=MM_DT, h=H, i_dim=I, tpc=TPC, compile=True):
    """One SPMD program: router shard + expert MLP at capacity cp."""
    chunks = _chunks_for(cp)
    kt_h = h // 128            # k-tiles over H
    mt_i = i_dim // 128        # m-tiles over I
    ht_h = h // 128            # h-tiles over H
    nc = bacc.Bacc(None, target_bir_lowering=False, debug=False)

    mdt = mm_dt                      # storage dtype for matmul operands
    xgt_d = nc.dram_tensor("xgt", [h, cp], mdt, kind="ExternalInput")
    wg_d = nc.dram_tensor("wg", [h, i_dim], mdt, kind="ExternalInput")
    wu_d = nc.dram_tensor("wu", [h, i_dim], mdt, kind="ExternalInput")
    wd_d = nc.dram_tensor("wd", [i_dim, h], BF16, kind="ExternalInput")
    sc_d = nc.dram_tensor("scale", [128, cp], F32, kind="ExternalInput")
    xrt_d = nc.dram_tensor("xrt", [h, tpc], F32, kind="ExternalInput")
    rwt_d = nc.dram_tensor("rwt", [h, E], F32, kind="ExternalInput")

    yt_d = nc.dram_tensor("yt", [h, cp], F32, kind="ExternalOutput")
    rlog_d = nc.dram_tensor("rlog", [tpc, E], F32, kind="ExternalOutput")
    rtopw_d = nc.dram_tensor("rtopw", [tpc, K], F32, kind="ExternalOutput")
    rtopi_d = nc.dram_tensor("rtopi", [tpc, K], I32, kind="ExternalOutput")

    with tile.TileContext(nc) as tc:
        # ---------------- router (tiny, fp32-exact) ----------------
        with (
            tc.tile_pool(name="rpool", bufs=2) as rpool,
            tc.tile_pool(name="rconst", bufs=1) as rconst,
            tc.tile_pool(name="rpsum", bufs=2, space="PSUM") as rpsum,
        ):
            zb = rconst.tile([128, 1], F32)
            nc.gpsimd.memset(zb[:], 0.0)
            rw_t = rconst.tile([128, kt_h, E], F32)
            nc.sync.dma_start(
                out=rw_t[:], in_=rwt_d.ap().rearrange("(k p) e -> p k e", p=128)
            )
            for mt in range(tpc // 128):
                xr_t = rpool.tile([128, kt_h, 128], F32, tag="xr")
                nc.sync.dma_start(
                    out=xr_t[:],
                    in_=xrt_d.ap()[:, mt * 128:(mt + 1) * 128].rearrange(
                        "(k p) t -> p k t", p=128
                    ),
                )
                ps = rpsum.tile([128, E], F32, tag="ps")
                for k in range(kt_h):
                    nc.tensor.matmul(
                        ps[:], xr_t[:, k, :], rw_t[:, k, :],
                        start=(k == 0), stop=(k == kt_h - 1),
                    )
                lg = rpool.tile([128, E], F32, tag="lg")
                nc.vector.tensor_copy(lg[:], ps[:])
                nc.sync.dma_start(out=rlog_d[mt * 128:(mt + 1) * 128, :], in_=lg[:])

                mx = rpool.tile([128, 8], F32, tag="mx")
                nc.vector.max(mx[:], lg[:])
                mi = rpool.tile([128, 8], U32, tag="mi")
                nc.vector.max_index(mi[:], mx[:], lg[:])

                d21 = rpool.tile([128, 1], F32, tag="d21")
                nc.vector.tensor_tensor(
                    d21[:], mx[:, 1:2], mx[:, 0:1], mybir.AluOpType.subtract
                )
                pw = rpool.tile([128, K], F32, tag="pw")
                # top_w = (sigma(l1-l2), sigma(l2-l1)) — exact softmax-renorm pair
                nc.scalar.activation(
                    pw[:, 0:1], d21[:], mybir.ActivationFunctionType.Sigmoid,
                    bias=zb[:], scale=-1.0,
                )
                nc.scalar.activation(
                    pw[:, 1:2], d21[:], mybir.ActivationFunctionType.Sigmoid,
                    bias=zb[:], scale=1.0,
                )
                nc.sync.dma_start(out=rtopw_d[mt * 128:(mt + 1) * 128, :], in_=pw[:])

                ti = rpool.tile([128, K], I32, tag="ti")
                nc.vector.tensor_copy(ti[:], mi[:, 0:K])
                nc.sync.dma_start(out=rtopi_d[mt * 128:(mt + 1) * 128, :], in_=ti[:])

        # ---------------- expert MLP ----------------
        with (
            tc.tile_pool(name="xgp", bufs=1) as xgp,
            tc.tile_pool(name="wgup", bufs=2) as wgup,
            tc.tile_pool(name="actp", bufs=2) as actp,
            tc.tile_pool(name="wdp", bufs=2) as wdp,
            tc.tile_pool(name="stage", bufs=2) as stage,
            tc.tile_pool(name="scp", bufs=1) as scp,
            tc.tile_pool(name="mpsum", bufs=2, space="PSUM") as mpsum,
        ):
            zb2 = scp.tile([128, 1], F32, tag="zb2")
            nc.gpsimd.memset(zb2[:], 0.0)
            for off, csz in chunks:
                xg = xgp.tile([128, kt_h, csz], mdt, tag="xg")
                nc.sync.dma_start(
                    out=xg[:],
                    in_=xgt_d.ap()[:, off:off + csz].rearrange(
                        "(k p) c -> p k c", p=128
                    ),
                )
                sct = scp.tile([128, csz], F32, tag="sc")
                nc.gpsimd.dma_start(out=sct[:], in_=sc_d.ap()[:, off:off + csz])

                act = actp.tile([128, mt_i, csz], BF16, tag="act")
                for m in range(mt_i):
                    wgt = wgup.tile([128, kt_h, 128], mdt, tag="wg")
                    nc.sync.dma_start(
                        out=wgt[:],
                        in_=wg_d.ap()[:, m * 128:(m + 1) * 128].rearrange(
                            "(k p) m -> p k m", p=128
                        ),
                    )
                    wut = wgup.tile([128, kt_h, 128], mdt, tag="wu")
                    nc.scalar.dma_start(
                        out=wut[:],
                        in_=wu_d.ap()[:, m * 128:(m + 1) * 128].rearrange(
                            "(k p) m -> p k m", p=128
                        ),
                    )
                    pg = mpsum.tile([128, csz], F32, tag="pg")
                    for k in range(kt_h):
                        nc.tensor.matmul(
                            pg[:], wgt[:, k, :], xg[:, k, :],
                            start=(k == 0), stop=(k == kt_h - 1),
                        )
                    pu = mpsum.tile([128, csz], F32, tag="pu")
                    for k in range(kt_h):
                        nc.tensor.matmul(
                            pu[:], wut[:, k, :], xg[:, k, :],
                            start=(k == 0), stop=(k == kt_h - 1),
                        )
                    nc.scalar.activation(
                        act[:, m, :], pg[:], mybir.ActivationFunctionType.Silu,
                        bias=zb2[:],
                    )
                    nc.vector.tensor_tensor(
                        act[:, m, :], act[:, m, :], pu[:], mybir.AluOpType.mult
                    )

                for hh in range(ht_h):
                    py = mpsum.tile([128, csz], F32, tag="py")
                    nhalf = 2 if mt_i % 2 == 0 else 1
                    for half in range(nhalf):
                        k0 = half * (mt_i // nhalf)
                        k1 = (half + 1) * (mt_i // nhalf) if half < nhalf - 1 else mt_i
                        wdt = wdp.tile([128, k1 - k0, 128], BF16, tag="wd")
                        nc.gpsimd.dma_start(
                            out=wdt[:],
                            in_=wd_d.ap()[
                                k0 * 128:k1 * 128, hh * 128:(hh + 1) * 128
                            ].rearrange("(k p) m -> p k m", p=128),
                        )
                        for kk in range(k1 - k0):
                            k = k0 + kk
                            nc.tensor.matmul(
                                py[:], wdt[:, kk, :], act[:, k, :],
                                start=(k == 0), stop=(k == mt_i - 1),
                            )
                    yt_sb = stage.tile([128, csz], F32, tag="y")
                    nc.vector.tensor_tensor(
                        yt_sb[:], py[:], sct[:], mybir.AluOpType.mult
                    )
                    nc.scalar.dma_start(
                        out=yt_d[hh * 128:(hh + 1) * 128, off:off + csz], in_=yt_sb[:]
                    )
    if compile:
        nc.compile()
    return nc


def _route_host(x: np.ndarray, router_w: np.ndarray):
    """Dispatch-only routing decisions (must order-match the fp32 reference)."""
    logits = x @ router_w.T                      # [T, E] fp32
    order = np.argsort(-logits, axis=1, kind="stable")
    top2 = order[:, :K].astype(np.int64)          # ties -> lower index, like jax
    l12 = np.take_along_axis(logits, top2, axis=1).astype(np.float64)
    d = l12[:, 0] - l12[:, 1]
    w1 = 1.0 / (1.0 + np.exp(-d))
    topw = np.stack([w1, 1.0 - w1], axis=1)       # renormalized top-2 weights
    return top2, topw


def kernel(hidden_states, router_w, w_gate, w_up, w_down):
    global LAST_RESULT
    x = np.ascontiguousarray(
        np.asarray(hidden_states, dtype=np.float32).reshape(T, H)
    )
    router_w = np.asarray(router_w, dtype=np.float32)
    w_gate = np.ascontiguousarray(np.asarray(w_gate, dtype=np.float32))
    w_up = np.ascontiguousarray(np.asarray(w_up, dtype=np.float32))
    w_down = np.ascontiguousarray(np.asarray(w_down, dtype=np.float32))

    top2, topw = _route_host(x, router_w)

    counts = np.bincount(top2.ravel(), minlength=E)
    cp = round_capacity(int(counts.max()))

    rows_per_e = []
    flat_idx = np.empty((T, K), dtype=np.int64)
    scales = np.zeros((E, cp), dtype=np.float32)
    for e in range(E):
        m0 = top2[:, 0] == e
        m1 = top2[:, 1] == e
        rows = np.nonzero(m0 | m1)[0]
        rows_per_e.append(rows)
        pos = np.arange(len(rows), dtype=np.int64)
        slot0 = m0[rows]                  # which slot routed each row here
        flat_idx[rows[slot0], 0] = e * cp + pos[slot0]
        flat_idx[rows[~slot0], 1] = e * cp + pos[~slot0]
        scales[e, :len(rows)] = np.where(
            slot0, topw[rows, 0], topw[rows, 1]
        ).astype(np.float32)

    rwt = np.ascontiguousarray(router_w.T)

    in_maps = []
    for e in range(NCORES):
        rows = rows_per_e[e]
        xgt = np.zeros((H, cp), dtype=np.float32)
        xgt[:, :len(rows)] = x[rows].T
        in_maps.append({
            "xgt": xgt,
            "wg": w_gate[e],
            "wu": w_up[e],
            "wd": w_down[e].astype(_ml.bfloat16),
            "scale": np.ascontiguousarray(
                np.broadcast_to(scales[e], (128, cp))
            ),
            "xrt": np.ascontiguousarray(x[e * TPC:(e + 1) * TPC].T),
            "rwt": rwt,
        })

    nc = build_moe_program(cp)
    res = run_bass_kernel_spmd(nc, in_maps, list(range(NCORES)), trace=TRACE)
    LAST_RESULT = res
    results = res.results

    yflat = np.concatenate(
        [results[e]["yt"].T for e in range(E)], axis=0
    )  # [E*cp, H]
    out = yflat[flat_idx[:, 0]] + yflat[flat_idx[:, 1]]

    router_logits = np.concatenate(
        [results[c]["rlog"] for c in range(NCORES)], axis=0
    )
    top_i = np.concatenate(
        [results[c]["rtopi"] for c in range(NCORES)], axis=0
    ).astype(np.int32)
    top_w = np.concatenate(
        [results[c]["rtopw"] for c in range(NCORES)], axis=0
    )

    return (
        out.reshape(B, S, H),
        router_logits.reshape(B, S, E),
        top_i.reshape(B, S, K),
        top_w.reshape(B, S, K),
    )


# revision 11
# speedup vs baseline: 1.2343x; 1.2014x over previous
"""Trainium2 Bass kernel for a Llama-style MoE layer (8 experts, top-2).

Strategy (8 NeuronCores, SPMD):
  - Expert-parallel: core e owns expert e's weights (w_gate/w_up/w_down[e]).
  - Host computes router logits once (tiny: [T,H]@[H,E]) purely to DECIDE
    dispatch; tokens routed to expert e are gathered, padded to capacity CP,
    and shipped transposed as xgt=[H,CP] to core e.
  - Device, per core, in one SPMD launch:
      * router for its 1/8 token shard: logits = x_shard @ router_w.T (fp32
        matmuls), top-2 via vector max/max_index, renormalized top-2 weights
        via sigmoid(l1-l2) — produces router_logits/top_i/top_w outputs.
      * expert SwiGLU MLP on its gathered tokens, entirely in
        [feature-partition, token-free] layout so no transposes are needed:
            gT[i,c] += wg[k,i].T @ xgt[k,c]   (accum over k: H-tiles)
            aT = silu(gT) * uT
            yT[h,c] += wd[k,h].T @ aT[k,c]    (accum over k: I-tiles)
        then yT scaled by the token's combine weight, DMA'd out.
  - Host scatters: out[t] = y[e1(t), pos1(t)] + y[e2(t), pos2(t)].

The MLP matmuls run as float32r (full fp32 storage; PE reduced-precision
single-pass mode, 1 cycle/row at N>=256 vs 4 for plain fp32).
"""

import sys

for _p in ("/opt/trn_rl_repo", "/root/.axon_site/_ro/trn_rl_repo"):
    if _p not in sys.path:
        sys.path.append(_p)

import numpy as np
import ml_dtypes as _ml

from concourse import bacc, bass, mybir, tile
from concourse import bass_utils
from concourse.bass_utils import run_bass_kernel_spmd

# zero-egress container: don't ship NEFF/NTFF dirs to a bucket when tracing
bass_utils.upload_artifacts = lambda tmpdir: "local://" + tmpdir

B, S, H, I, E, K = 4, 2048, 2048, 5632, 8, 2
T = B * S
NCORES = 8
TPC = T // NCORES          # router tokens per core

F32 = mybir.dt.float32
F32R = mybir.dt.float32r
BF16 = mybir.dt.bfloat16
I32 = mybir.dt.int32
U32 = mybir.dt.uint32

MM_DT = F32R               # matmul compute dtype for the expert MLP
TRACE = False              # set by test harness to capture an NTFF profile
LAST_RESULT = None         # harness introspection: last BassKernelResults


def _chunks_for(cp: int) -> list[tuple[int, int]]:
    """Split capacity into free-dim chunks: full 512s plus a >=256 remainder."""
    assert cp % 128 == 0 and cp >= 256
    out = []
    off = 0
    while cp - off > 512:
        rem = cp - off
        if rem - 512 == 128:             # would leave a 128 tail: emit 384 now
            out.append((off, 384))
            off += 384
        else:
            out.append((off, 512))
            off += 512
    out.append((off, cp - off))
    assert sum(c for _, c in out) == cp
    assert all(c in (256, 384, 512) for _, c in out)
    return out


def round_capacity(n: int) -> int:
    cp = max(256, ((n + 127) // 128) * 128)
    if cp % 512 == 128:   # avoid a 128 tail chunk
        cp += 128
    return cp


def build_moe_program(cp, mm_dt=MM_DT, h=H, i_dim=I, tpc=TPC, compile=True):
    """One SPMD program: router shard + expert MLP at capacity cp."""
    chunks = _chunks_for(cp)
    kt_h = h // 128            # k-tiles over H
    mt_i = i_dim // 128        # m-tiles over I
    ht_h = h // 128            # h-tiles over H
    nc = bacc.Bacc(None, target_bir_lowering=False, debug=False)

    mdt = mm_dt                      # storage dtype for matmul operands
    xgt_d = nc.dram_tensor("xgt", [h, cp], mdt, kind="ExternalInput")
    wg_d = nc.dram_tensor("wg", [h, i_dim], mdt, kind="ExternalInput")
    wu_d = nc.dram_tensor("wu", [h, i_dim], mdt, kind="ExternalInput")
    wd_d = nc.dram_tensor("wd", [i_dim, h], BF16, kind="ExternalInput")
    sc_d = nc.dram_tensor("scale", [128, cp], F32, kind="ExternalInput")
    xrt_d = nc.dram_tensor("xrt", [h, tpc], F32, kind="ExternalInput")
    rwt_d = nc.dram_tensor("rwt", [h, E], F32, kind="ExternalInput")

    yt_d = nc.dram_tensor("yt", [h, cp], F32, kind="ExternalOutput")
    rlog_d = nc.dram_tensor("rlog", [tpc, E], F32, kind="ExternalOutput")
    rtopw_d = nc.dram_tensor("rtopw", [tpc, K], F32, kind="ExternalOutput")
    rtopi_d = nc.dram_tensor("rtopi", [tpc, K], I32, kind="ExternalOutput")

    with tile.TileContext(nc) as tc:
        # ---------------- expert MLP ----------------
        with (
            tc.tile_pool(name="xgp", bufs=1) as xgp,
            tc.tile_pool(name="wgup", bufs=2) as wgup,
            tc.tile_pool(name="actp", bufs=2) as actp,
            tc.tile_pool(name="wdp", bufs=4) as wdp,
            tc.tile_pool(name="stage", bufs=2) as stage,
            tc.tile_pool(name="scp", bufs=1) as scp,
            tc.tile_pool(name="mpsum", bufs=2, space="PSUM") as mpsum,
        ):
            zb2 = scp.tile([128, 1], F32, tag="zb2")
            nc.gpsimd.memset(zb2[:], 0.0)
            for off, csz in chunks:
                xg = xgp.tile([128, kt_h, csz], mdt, tag="xg")
                nc.sync.dma_start(
                    out=xg[:],
                    in_=xgt_d.ap()[:, off:off + csz].rearrange(
                        "(k p) c -> p k c", p=128
                    ),
                )
                sct = scp.tile([128, csz], F32, tag="sc")
                nc.gpsimd.dma_start(out=sct[:], in_=sc_d.ap()[:, off:off + csz])

                act = actp.tile([128, mt_i, csz], BF16, tag="act")
                for m in range(mt_i):
                    wgt = wgup.tile([128, kt_h, 128], mdt, tag="wg")
                    nc.sync.dma_start(
                        out=wgt[:],
                        in_=wg_d.ap()[:, m * 128:(m + 1) * 128].rearrange(
                            "(k p) m -> p k m", p=128
                        ),
                    )
                    wut = wgup.tile([128, kt_h, 128], mdt, tag="wu")
                    nc.scalar.dma_start(
                        out=wut[:],
                        in_=wu_d.ap()[:, m * 128:(m + 1) * 128].rearrange(
                            "(k p) m -> p k m", p=128
                        ),
                    )
                    pg = mpsum.tile([128, csz], F32, tag="pg")
                    for k in range(kt_h):
                        nc.tensor.matmul(
                            pg[:], wgt[:, k, :], xg[:, k, :],
                            start=(k == 0), stop=(k == kt_h - 1),
                        )
                    pu = mpsum.tile([128, csz], F32, tag="pu")
                    for k in range(kt_h):
                        nc.tensor.matmul(
                            pu[:], wut[:, k, :], xg[:, k, :],
                            start=(k == 0), stop=(k == kt_h - 1),
                        )
                    nc.scalar.activation(
                        act[:, m, :], pg[:], mybir.ActivationFunctionType.Silu,
                        bias=zb2[:],
                    )
                    nc.vector.tensor_tensor(
                        act[:, m, :], act[:, m, :], pu[:], mybir.AluOpType.mult
                    )

                for hh in range(ht_h):
                    py = mpsum.tile([128, csz], F32, tag="py")
                    nquart = 4 if mt_i % 4 == 0 else (2 if mt_i % 2 == 0 else 1)
                    for half in range(nquart):
                        k0 = half * (mt_i // nquart)
                        k1 = (half + 1) * (mt_i // nquart) if half < nquart - 1 else mt_i
                        wdt = wdp.tile([128, k1 - k0, 128], BF16, tag="wd")
                        nc.sync.dma_start(
                            out=wdt[:],
                            in_=wd_d.ap()[
                                k0 * 128:k1 * 128, hh * 128:(hh + 1) * 128
                            ].rearrange("(k p) m -> p k m", p=128),
                        )
                        for kk in range(k1 - k0):
                            k = k0 + kk
                            nc.tensor.matmul(
                                py[:], wdt[:, kk, :], act[:, k, :],
                                start=(k == 0), stop=(k == mt_i - 1),
                            )
                    yt_sb = stage.tile([128, csz], F32, tag="y")
                    nc.vector.tensor_tensor(
                        yt_sb[:], py[:], sct[:], mybir.AluOpType.mult
                    )
                    nc.scalar.dma_start(
                        out=yt_d[hh * 128:(hh + 1) * 128, off:off + csz], in_=yt_sb[:]
                    )
        # ---------------- router (tiny, fp32-exact) ----------------
        with (
            tc.tile_pool(name="rpool", bufs=2) as rpool,
            tc.tile_pool(name="rconst", bufs=1) as rconst,
            tc.tile_pool(name="rpsum", bufs=2, space="PSUM") as rpsum,
        ):
            zb = rconst.tile([128, 1], F32)
            nc.gpsimd.memset(zb[:], 0.0)
            rw_t = rconst.tile([128, kt_h, E], F32)
            nc.sync.dma_start(
                out=rw_t[:], in_=rwt_d.ap().rearrange("(k p) e -> p k e", p=128)
            )
            for mt in range(tpc // 128):
                xr_t = rpool.tile([128, kt_h, 128], F32, tag="xr")
                nc.sync.dma_start(
                    out=xr_t[:],
                    in_=xrt_d.ap()[:, mt * 128:(mt + 1) * 128].rearrange(
                        "(k p) t -> p k t", p=128
                    ),
                )
                ps = rpsum.tile([128, E], F32, tag="ps")
                for k in range(kt_h):
                    nc.tensor.matmul(
                        ps[:], xr_t[:, k, :], rw_t[:, k, :],
                        start=(k == 0), stop=(k == kt_h - 1),
                    )
                lg = rpool.tile([128, E], F32, tag="lg")
                nc.vector.tensor_copy(lg[:], ps[:])
                nc.sync.dma_start(out=rlog_d[mt * 128:(mt + 1) * 128, :], in_=lg[:])

                mx = rpool.tile([128, 8], F32, tag="mx")
                nc.vector.max(mx[:], lg[:])
                mi = rpool.tile([128, 8], U32, tag="mi")
                nc.vector.max_index(mi[:], mx[:], lg[:])

                d21 = rpool.tile([128, 1], F32, tag="d21")
                nc.vector.tensor_tensor(
                    d21[:], mx[:, 1:2], mx[:, 0:1], mybir.AluOpType.subtract
                )
                pw = rpool.tile([128, K], F32, tag="pw")
                # top_w = (sigma(l1-l2), sigma(l2-l1)) — exact softmax-renorm pair
                nc.scalar.activation(
                    pw[:, 0:1], d21[:], mybir.ActivationFunctionType.Sigmoid,
                    bias=zb[:], scale=-1.0,
                )
                nc.scalar.activation(
                    pw[:, 1:2], d21[:], mybir.ActivationFunctionType.Sigmoid,
                    bias=zb[:], scale=1.0,
                )
                nc.sync.dma_start(out=rtopw_d[mt * 128:(mt + 1) * 128, :], in_=pw[:])

                ti = rpool.tile([128, K], I32, tag="ti")
                nc.vector.tensor_copy(ti[:], mi[:, 0:K])
                nc.sync.dma_start(out=rtopi_d[mt * 128:(mt + 1) * 128, :], in_=ti[:])

    if compile:
        nc.compile()
    return nc


def _route_host(x: np.ndarray, router_w: np.ndarray):
    """Dispatch-only routing decisions (must order-match the fp32 reference)."""
    logits = x @ router_w.T                      # [T, E] fp32
    order = np.argsort(-logits, axis=1, kind="stable")
    top2 = order[:, :K].astype(np.int64)          # ties -> lower index, like jax
    l12 = np.take_along_axis(logits, top2, axis=1).astype(np.float64)
    d = l12[:, 0] - l12[:, 1]
    w1 = 1.0 / (1.0 + np.exp(-d))
    topw = np.stack([w1, 1.0 - w1], axis=1)       # renormalized top-2 weights
    return top2, topw


def kernel(hidden_states, router_w, w_gate, w_up, w_down):
    global LAST_RESULT
    x = np.ascontiguousarray(
        np.asarray(hidden_states, dtype=np.float32).reshape(T, H)
    )
    router_w = np.asarray(router_w, dtype=np.float32)
    w_gate = np.ascontiguousarray(np.asarray(w_gate, dtype=np.float32))
    w_up = np.ascontiguousarray(np.asarray(w_up, dtype=np.float32))
    w_down = np.ascontiguousarray(np.asarray(w_down, dtype=np.float32))

    top2, topw = _route_host(x, router_w)

    counts = np.bincount(top2.ravel(), minlength=E)
    cp = round_capacity(int(counts.max()))

    rows_per_e = []
    flat_idx = np.empty((T, K), dtype=np.int64)
    scales = np.zeros((E, cp), dtype=np.float32)
    for e in range(E):
        m0 = top2[:, 0] == e
        m1 = top2[:, 1] == e
        rows = np.nonzero(m0 | m1)[0]
        rows_per_e.append(rows)
        pos = np.arange(len(rows), dtype=np.int64)
        slot0 = m0[rows]                  # which slot routed each row here
        flat_idx[rows[slot0], 0] = e * cp + pos[slot0]
        flat_idx[rows[~slot0], 1] = e * cp + pos[~slot0]
        scales[e, :len(rows)] = np.where(
            slot0, topw[rows, 0], topw[rows, 1]
        ).astype(np.float32)

    rwt = np.ascontiguousarray(router_w.T)

    mm_np = _ml.bfloat16 if MM_DT == BF16 else np.float32
    in_maps = []
    for e in range(NCORES):
        rows = rows_per_e[e]
        xgt = np.zeros((H, cp), dtype=np.float32)
        xgt[:, :len(rows)] = x[rows].T
        in_maps.append({
            "xgt": xgt.astype(mm_np, copy=False),
            "wg": w_gate[e].astype(mm_np, copy=False),
            "wu": w_up[e].astype(mm_np, copy=False),
            "wd": w_down[e].astype(_ml.bfloat16),
            "scale": np.ascontiguousarray(
                np.broadcast_to(scales[e], (128, cp))
            ),
            "xrt": np.ascontiguousarray(x[e * TPC:(e + 1) * TPC].T),
            "rwt": rwt,
        })

    nc = build_moe_program(cp)
    res = run_bass_kernel_spmd(nc, in_maps, list(range(NCORES)), trace=TRACE)
    LAST_RESULT = res
    results = res.results

    yflat = np.concatenate(
        [results[e]["yt"].T for e in range(E)], axis=0
    )  # [E*cp, H]
    out = yflat[flat_idx[:, 0]] + yflat[flat_idx[:, 1]]

    router_logits = np.concatenate(
        [results[c]["rlog"] for c in range(NCORES)], axis=0
    )
    top_i = np.concatenate(
        [results[c]["rtopi"] for c in range(NCORES)], axis=0
    ).astype(np.int32)
    top_w = np.concatenate(
        [results[c]["rtopw"] for c in range(NCORES)], axis=0
    )

    return (
        out.reshape(B, S, H),
        router_logits.reshape(B, S, E),
        top_i.reshape(B, S, K),
        top_w.reshape(B, S, K),
    )


# revision 12
# speedup vs baseline: 1.3415x; 1.0868x over previous
"""Trainium2 Bass kernel for a Llama-style MoE layer (8 experts, top-2).

Strategy (8 NeuronCores, SPMD):
  - Expert-parallel: core e owns expert e's weights (w_gate/w_up/w_down[e]).
  - Host computes router logits once (tiny: [T,H]@[H,E]) purely to DECIDE
    dispatch; tokens routed to expert e are gathered, padded to capacity CP,
    and shipped transposed as xgt=[H,CP] to core e.
  - Device, per core, in one SPMD launch:
      * router for its 1/8 token shard: logits = x_shard @ router_w.T (fp32
        matmuls), top-2 via vector max/max_index, renormalized top-2 weights
        via sigmoid(l1-l2) — produces router_logits/top_i/top_w outputs.
      * expert SwiGLU MLP on its gathered tokens, entirely in
        [feature-partition, token-free] layout so no transposes are needed:
            gT[i,c] += wg[k,i].T @ xgt[k,c]   (accum over k: H-tiles)
            aT = silu(gT) * uT
            yT[h,c] += wd[k,h].T @ aT[k,c]    (accum over k: I-tiles)
        then yT scaled by the token's combine weight, DMA'd out.
  - Host scatters: out[t] = y[e1(t), pos1(t)] + y[e2(t), pos2(t)].

The MLP matmuls run as float32r (full fp32 storage; PE reduced-precision
single-pass mode, 1 cycle/row at N>=256 vs 4 for plain fp32).
"""

import sys

for _p in ("/opt/trn_rl_repo", "/root/.axon_site/_ro/trn_rl_repo"):
    if _p not in sys.path:
        sys.path.append(_p)

import numpy as np
import ml_dtypes as _ml

from concourse import bacc, bass, mybir, tile
from concourse import bass_utils
from concourse.bass_utils import run_bass_kernel_spmd

# zero-egress container: don't ship NEFF/NTFF dirs to a bucket when tracing
bass_utils.upload_artifacts = lambda tmpdir: "local://" + tmpdir

B, S, H, I, E, K = 4, 2048, 2048, 5632, 8, 2
T = B * S
NCORES = 8
TPC = T // NCORES          # router tokens per core

F32 = mybir.dt.float32
F32R = mybir.dt.float32r
BF16 = mybir.dt.bfloat16
I32 = mybir.dt.int32
U32 = mybir.dt.uint32

MM_DT = F32R               # matmul compute dtype for the expert MLP
TRACE = False              # set by test harness to capture an NTFF profile
LAST_RESULT = None         # harness introspection: last BassKernelResults


def _chunks_for(cp: int) -> list[tuple[int, int]]:
    """Split capacity into free-dim chunks: full 512s plus a >=256 remainder."""
    assert cp % 128 == 0 and cp >= 256
    out = []
    off = 0
    while cp - off > 512:
        rem = cp - off
        if rem - 512 == 128:             # would leave a 128 tail: emit 384 now
            out.append((off, 384))
            off += 384
        else:
            out.append((off, 512))
            off += 512
    out.append((off, cp - off))
    assert sum(c for _, c in out) == cp
    assert all(c in (256, 384, 512) for _, c in out)
    return out


def round_capacity(n: int) -> int:
    cp = max(256, ((n + 127) // 128) * 128)
    if cp % 512 == 128:   # avoid a 128 tail chunk
        cp += 128
    return cp


def build_moe_program(cp, mm_dt=MM_DT, h=H, i_dim=I, tpc=TPC, compile=True):
    """One SPMD program: router shard + expert MLP at capacity cp."""
    chunks = _chunks_for(cp)
    kt_h = h // 128            # k-tiles over H
    mt_i = i_dim // 128        # m-tiles over I
    ht_h = h // 128            # h-tiles over H
    nc = bacc.Bacc(None, target_bir_lowering=False, debug=False)

    mdt = mm_dt                      # storage dtype for matmul operands
    xgt_d = nc.dram_tensor("xgt", [h, cp], mdt, kind="ExternalInput")
    wg_d = nc.dram_tensor("wg", [h, i_dim], mdt, kind="ExternalInput")
    wu_d = nc.dram_tensor("wu", [h, i_dim], mdt, kind="ExternalInput")
    wd_d = nc.dram_tensor("wd", [i_dim, h], BF16, kind="ExternalInput")
    sc_d = nc.dram_tensor("scale", [128, cp], F32, kind="ExternalInput")
    xrt_d = nc.dram_tensor("xrt", [h, tpc], F32, kind="ExternalInput")
    rwt_d = nc.dram_tensor("rwt", [h, E], F32, kind="ExternalInput")

    yt_d = nc.dram_tensor("yt", [h, cp], F32, kind="ExternalOutput")
    rlog_d = nc.dram_tensor("rlog", [tpc, E], F32, kind="ExternalOutput")
    rtopw_d = nc.dram_tensor("rtopw", [tpc, K], F32, kind="ExternalOutput")
    rtopi_d = nc.dram_tensor("rtopi", [tpc, K], I32, kind="ExternalOutput")

    with tile.TileContext(nc) as tc:
        # ---------------- expert MLP ----------------
        with (
            tc.tile_pool(name="xgp", bufs=1) as xgp,
            tc.tile_pool(name="wgup", bufs=2) as wgup,
            tc.tile_pool(name="actp", bufs=2) as actp,
            tc.tile_pool(name="wdp", bufs=4) as wdp,
            tc.tile_pool(name="stage", bufs=2) as stage,
            tc.tile_pool(name="scp", bufs=1) as scp,
            tc.tile_pool(name="mpsum", bufs=2, space="PSUM") as mpsum,
        ):
            zb2 = scp.tile([128, 1], F32, tag="zb2")
            nc.gpsimd.memset(zb2[:], 0.0)
            for off, csz in chunks:
                xg = xgp.tile([128, kt_h, csz], mdt, tag="xg")
                nc.gpsimd.dma_start(
                    out=xg[:],
                    in_=xgt_d.ap()[:, off:off + csz].rearrange(
                        "(k p) c -> p k c", p=128
                    ),
                )
                sct = scp.tile([128, csz], F32, tag="sc")
                nc.gpsimd.dma_start(out=sct[:], in_=sc_d.ap()[:, off:off + csz])

                act = actp.tile([128, mt_i, csz], BF16, tag="act")
                for m in range(mt_i):
                    wgt = wgup.tile([128, kt_h, 128], mdt, tag="wg")
                    nc.sync.dma_start(
                        out=wgt[:],
                        in_=wg_d.ap()[:, m * 128:(m + 1) * 128].rearrange(
                            "(k p) m -> p k m", p=128
                        ),
                    )
                    wut = wgup.tile([128, kt_h, 128], mdt, tag="wu")
                    nc.scalar.dma_start(
                        out=wut[:],
                        in_=wu_d.ap()[:, m * 128:(m + 1) * 128].rearrange(
                            "(k p) m -> p k m", p=128
                        ),
                    )
                    pg = mpsum.tile([128, csz], F32, tag="pg")
                    for k in range(kt_h):
                        nc.tensor.matmul(
                            pg[:], wgt[:, k, :], xg[:, k, :],
                            start=(k == 0), stop=(k == kt_h - 1),
                        )
                    pu = mpsum.tile([128, csz], F32, tag="pu")
                    for k in range(kt_h):
                        nc.tensor.matmul(
                            pu[:], wut[:, k, :], xg[:, k, :],
                            start=(k == 0), stop=(k == kt_h - 1),
                        )
                    nc.scalar.activation(
                        act[:, m, :], pg[:], mybir.ActivationFunctionType.Silu,
                        bias=zb2[:],
                    )
                    nc.vector.tensor_tensor(
                        act[:, m, :], act[:, m, :], pu[:], mybir.AluOpType.mult
                    )

                for hh in range(ht_h):
                    py = mpsum.tile([128, csz], F32, tag="py")
                    nquart = 4 if mt_i % 4 == 0 else (2 if mt_i % 2 == 0 else 1)
                    for half in range(nquart):
                        k0 = half * (mt_i // nquart)
                        k1 = (half + 1) * (mt_i // nquart) if half < nquart - 1 else mt_i
                        wdt = wdp.tile([128, k1 - k0, 128], BF16, tag="wd")
                        nc.sync.dma_start(
                            out=wdt[:],
                            in_=wd_d.ap()[
                                k0 * 128:k1 * 128, hh * 128:(hh + 1) * 128
                            ].rearrange("(k p) m -> p k m", p=128),
                        )
                        for kk in range(k1 - k0):
                            k = k0 + kk
                            nc.tensor.matmul(
                                py[:], wdt[:, kk, :], act[:, k, :],
                                start=(k == 0), stop=(k == mt_i - 1),
                            )
                    yt_sb = stage.tile([128, csz], F32, tag="y")
                    nc.vector.tensor_tensor(
                        yt_sb[:], py[:], sct[:], mybir.AluOpType.mult
                    )
                    nc.scalar.dma_start(
                        out=yt_d[hh * 128:(hh + 1) * 128, off:off + csz], in_=yt_sb[:]
                    )
        # ---------------- router (tiny, fp32-exact) ----------------
        with (
            tc.tile_pool(name="rpool", bufs=2) as rpool,
            tc.tile_pool(name="rconst", bufs=1) as rconst,
            tc.tile_pool(name="rpsum", bufs=2, space="PSUM") as rpsum,
        ):
            zb = rconst.tile([128, 1], F32)
            nc.gpsimd.memset(zb[:], 0.0)
            rw_t = rconst.tile([128, kt_h, E], F32)
            nc.sync.dma_start(
                out=rw_t[:], in_=rwt_d.ap().rearrange("(k p) e -> p k e", p=128)
            )
            for mt in range(tpc // 128):
                xr_t = rpool.tile([128, kt_h, 128], F32, tag="xr")
                nc.sync.dma_start(
                    out=xr_t[:],
                    in_=xrt_d.ap()[:, mt * 128:(mt + 1) * 128].rearrange(
                        "(k p) t -> p k t", p=128
                    ),
                )
                ps = rpsum.tile([128, E], F32, tag="ps")
                for k in range(kt_h):
                    nc.tensor.matmul(
                        ps[:], xr_t[:, k, :], rw_t[:, k, :],
                        start=(k == 0), stop=(k == kt_h - 1),
                    )
                lg = rpool.tile([128, E], F32, tag="lg")
                nc.vector.tensor_copy(lg[:], ps[:])
                nc.sync.dma_start(out=rlog_d[mt * 128:(mt + 1) * 128, :], in_=lg[:])

                mx = rpool.tile([128, 8], F32, tag="mx")
                nc.vector.max(mx[:], lg[:])
                mi = rpool.tile([128, 8], U32, tag="mi")
                nc.vector.max_index(mi[:], mx[:], lg[:])

                d21 = rpool.tile([128, 1], F32, tag="d21")
                nc.vector.tensor_tensor(
                    d21[:], mx[:, 1:2], mx[:, 0:1], mybir.AluOpType.subtract
                )
                pw = rpool.tile([128, K], F32, tag="pw")
                # top_w = (sigma(l1-l2), sigma(l2-l1)) — exact softmax-renorm pair
                nc.scalar.activation(
                    pw[:, 0:1], d21[:], mybir.ActivationFunctionType.Sigmoid,
                    bias=zb[:], scale=-1.0,
                )
                nc.scalar.activation(
                    pw[:, 1:2], d21[:], mybir.ActivationFunctionType.Sigmoid,
                    bias=zb[:], scale=1.0,
                )
                nc.sync.dma_start(out=rtopw_d[mt * 128:(mt + 1) * 128, :], in_=pw[:])

                ti = rpool.tile([128, K], I32, tag="ti")
                nc.vector.tensor_copy(ti[:], mi[:, 0:K])
                nc.sync.dma_start(out=rtopi_d[mt * 128:(mt + 1) * 128, :], in_=ti[:])

    if compile:
        nc.compile()
    return nc


def _route_host(x: np.ndarray, router_w: np.ndarray):
    """Dispatch-only routing decisions (must order-match the fp32 reference)."""
    logits = x @ router_w.T                      # [T, E] fp32
    order = np.argsort(-logits, axis=1, kind="stable")
    top2 = order[:, :K].astype(np.int64)          # ties -> lower index, like jax
    l12 = np.take_along_axis(logits, top2, axis=1).astype(np.float64)
    d = l12[:, 0] - l12[:, 1]
    w1 = 1.0 / (1.0 + np.exp(-d))
    topw = np.stack([w1, 1.0 - w1], axis=1)       # renormalized top-2 weights
    return top2, topw


def kernel(hidden_states, router_w, w_gate, w_up, w_down):
    global LAST_RESULT
    x = np.ascontiguousarray(
        np.asarray(hidden_states, dtype=np.float32).reshape(T, H)
    )
    router_w = np.asarray(router_w, dtype=np.float32)
    w_gate = np.ascontiguousarray(np.asarray(w_gate, dtype=np.float32))
    w_up = np.ascontiguousarray(np.asarray(w_up, dtype=np.float32))
    w_down = np.ascontiguousarray(np.asarray(w_down, dtype=np.float32))

    top2, topw = _route_host(x, router_w)

    counts = np.bincount(top2.ravel(), minlength=E)
    cp = round_capacity(int(counts.max()))

    rows_per_e = []
    flat_idx = np.empty((T, K), dtype=np.int64)
    scales = np.zeros((E, cp), dtype=np.float32)
    for e in range(E):
        m0 = top2[:, 0] == e
        m1 = top2[:, 1] == e
        rows = np.nonzero(m0 | m1)[0]
        rows_per_e.append(rows)
        pos = np.arange(len(rows), dtype=np.int64)
        slot0 = m0[rows]                  # which slot routed each row here
        flat_idx[rows[slot0], 0] = e * cp + pos[slot0]
        flat_idx[rows[~slot0], 1] = e * cp + pos[~slot0]
        scales[e, :len(rows)] = np.where(
            slot0, topw[rows, 0], topw[rows, 1]
        ).astype(np.float32)

    rwt = np.ascontiguousarray(router_w.T)

    mm_np = _ml.bfloat16 if MM_DT == BF16 else np.float32
    in_maps = []
    for e in range(NCORES):
        rows = rows_per_e[e]
        xgt = np.zeros((H, cp), dtype=np.float32)
        xgt[:, :len(rows)] = x[rows].T
        in_maps.append({
            "xgt": xgt.astype(mm_np, copy=False),
            "wg": w_gate[e].astype(mm_np, copy=False),
            "wu": w_up[e].astype(mm_np, copy=False),
            "wd": w_down[e].astype(_ml.bfloat16),
            "scale": np.ascontiguousarray(
                np.broadcast_to(scales[e], (128, cp))
            ),
            "xrt": np.ascontiguousarray(x[e * TPC:(e + 1) * TPC].T),
            "rwt": rwt,
        })

    nc = build_moe_program(cp)
    res = run_bass_kernel_spmd(nc, in_maps, list(range(NCORES)), trace=TRACE)
    LAST_RESULT = res
    results = res.results

    yflat = np.concatenate(
        [results[e]["yt"].T for e in range(E)], axis=0
    )  # [E*cp, H]
    out = yflat[flat_idx[:, 0]] + yflat[flat_idx[:, 1]]

    router_logits = np.concatenate(
        [results[c]["rlog"] for c in range(NCORES)], axis=0
    )
    top_i = np.concatenate(
        [results[c]["rtopi"] for c in range(NCORES)], axis=0
    ).astype(np.int32)
    top_w = np.concatenate(
        [results[c]["rtopw"] for c in range(NCORES)], axis=0
    )

    return (
        out.reshape(B, S, H),
        router_logits.reshape(B, S, E),
        top_i.reshape(B, S, K),
        top_w.reshape(B, S, K),
    )
